# revision 9
# baseline (speedup 1.0000x reference)
"""DispLoss kernel for Trainium2 (8 NeuronCores, Bass/Tile) — v2.

Math
----
reference loss per pixel p (B*H*W total):
    target = w_idx - disp
    mask   = valid & (disp < 192)
    pos    = clip(target + 0.1*W, 0, 1.1*W) / (1.1*W/255)   in [0, 255)
    lb = floor(pos); hb = lb+1; wh = pos-lb
    ce     = -((1-wh)*logp[lb] + wh*logp[hb]),  logp = x - lse(x)
    logits_loss = sum(ce*mask)/msum;  coord_loss = sum(|coord-target|*mask)/msum

Since the soft-label weights sum to 1, ce = lse - ((1-wh)*x[lb] + wh*x[hb]).
The second term is an O(N) two-element gather -> computed on the host from
the raw fp32 logits, along with coord_loss and msum. The device only
computes sum_p mask_p * lse_p — a pure streaming log-sum-exp over the
(B,256,H,W) logits.

Device layout (per core: 48 of 384 H-rows => P = 2*48*1216 = 116736 pixels):
    pixel-major fp8(e3m4) upload xl[p, g*256 + c], p in [0,128) the pixel
    slot, g in [0,912) the pixel group, c the channel.
    - ACT: exp (fp8 -> bf16), one instruction per tile (the bottleneck pass)
    - DVE: 8-level pairwise fold tree along the free axis (bf16 tensor_tensor
      adds run in 2x DVE perf mode) -> per-pixel sumexp
    - ACT: Ln, DVE: mask-weighted accumulate -> (128,1) partials, DMA out.
"""

import os
import sys
from contextlib import ExitStack

import numpy as np
import ml_dtypes

for _p in ("/opt/trn_rl_repo", "/root/.axon_site/_ro/trn_rl_repo"):
    if os.path.isdir(_p) and _p not in sys.path:
        sys.path.insert(0, _p)

B, H, W = 2, 384, 1216
NBINS = 256
NCORES = 8
HC = H // NCORES                   # 48 rows per core
P = B * HC * W                     # 116736 pixels per core
G = P // 128                       # 912 pixel groups per core

# device tiling: gb pixel-groups per tile, pg of them exp'd on GpSimd via
# a Schraudolph bit-trick (the rest on the scalar engine's Exp table)
CFG = dict(G=G, C=NBINS, GB=48, PG=14)

# Schraudolph exp, scaled so the LOW u16 lane of the fp32 affine result is
# exactly the bf16 bit pattern of exp(x):
#   t = x*SCH_A + SCH_B  (fp32; t in [2^23, 2^24) so bits(t)=0x4B000000+m,
#   m = round(x*2^23/(65536*ln2) + 127*2^23/65536) < 2^16 = bf16_bits(e^x))
SCH_A = float(2.0**23 / np.log(2.0) / 65536.0)
SCH_B = 8404857.0


def build_program(cfg):
    import concourse.bacc as bacc
    import concourse.tile as tile
    from concourse import mybir

    AF = mybir.ActivationFunctionType
    OP = mybir.AluOpType
    f32 = mybir.dt.float32
    bf16 = mybir.dt.bfloat16
    f8 = mybir.dt.float8e3
    u32 = mybir.dt.uint32
    u16 = mybir.dt.uint16

    Gc, C, GBt = cfg["G"], cfg["C"], cfg["GB"]
    PGt = cfg.get("PG", 0)
    GA = GBt - PGt
    NT = Gc // GBt
    assert NT * GBt == Gc

    nc = bacc.Bacc("TRN2", target_bir_lowering=False)
    xl = nc.dram_tensor("xl", [128, Gc * C], f8, kind="ExternalInput")
    mk = nc.dram_tensor("mk", [128, Gc], bf16, kind="ExternalInput")
    outp = nc.dram_tensor("outp", [128, 1], f32, kind="ExternalOutput")

    with ExitStack() as ctx:
        tc = ctx.enter_context(tile.TileContext(nc))
        consts = ctx.enter_context(tc.tile_pool(name="consts", bufs=1))
        xpool = ctx.enter_context(tc.tile_pool(name="xpool", bufs=3))
        epool = ctx.enter_context(tc.tile_pool(name="epool", bufs=2))
        fpool = ctx.enter_context(tc.tile_pool(name="fpool", bufs=2))
        smalls = ctx.enter_context(tc.tile_pool(name="smalls", bufs=1))

        mkt = consts.tile([128, Gc], bf16)
        nc.sync.dma_start(out=mkt, in_=mk[:, :])
        se_all = smalls.tile([128, Gc], bf16)

        for t in range(NT):
            xt = xpool.tile([128, GBt * C], f8, tag="xt")
            nc.sync.dma_start(out=xt, in_=xl[:, t * GBt * C:(t + 1) * GBt * C])
            # exact exp on the activation engine for GA groups
            et = epool.tile([128, GA * C], bf16, tag="et")
            nc.scalar.activation(out=et, in_=xt[:, 0:GA * C], func=AF.Exp)
            # Schraudolph exp on GpSimd for the remaining PG groups: one
            # affine; the low u16 lane of each fp32 result is bf16(exp(x))
            if PGt:
                t32 = epool.tile([128, PGt * C], f32, tag="t32")
                nc.gpsimd.tensor_scalar(
                    t32, xt[:, GA * C:GBt * C], SCH_A, SCH_B,
                    OP.mult, OP.add)
            # pairwise fold tree over channels: 256 -> 1 (bf16, 2x DVE mode)
            half = C // 2
            f1 = fpool.tile([128, GBt * half], bf16, tag=f"f{half}")
            f1v = f1.rearrange("p (g c) -> p g c", c=half)
            ev = et.rearrange("p (g c) -> p g c", c=C)
            nc.vector.tensor_tensor(
                out=f1v[:, 0:GA], in0=ev[:, :, 0:half], in1=ev[:, :, half:C],
                op=OP.add)
            if PGt:
                # bf16 values live at even lanes of the bitcast f32 words
                pv = t32.bitcast(bf16).rearrange(
                    "p (g c two) -> p g c two", two=2, c=C)
                nc.vector.tensor_tensor(
                    out=f1v[:, GA:GBt], in0=pv[:, :, 0:half, 0],
                    in1=pv[:, :, half:C, 0], op=OP.add)
            src, width = f1, half
            while width > 1:
                half = width // 2
                sv = src.rearrange("p (g c) -> p g c", c=width)
                if half == 1:
                    dst = se_all[:, t * GBt:(t + 1) * GBt]
                    dv = dst.rearrange("p (g c) -> p g c", c=1)
                else:
                    dst = fpool.tile([128, GBt * half], bf16, tag=f"f{half}")
                    dv = dst.rearrange("p (g c) -> p g c", c=half)
                nc.vector.tensor_tensor(
                    out=dv, in0=sv[:, :, 0:half], in1=sv[:, :, half:width],
                    op=OP.add)
                src, width = dst, half

        # epilogue: lse = ln(sumexp); partials[p] = sum_g mask*lse
        lse = smalls.tile([128, Gc], f32)
        nc.scalar.activation(out=lse, in_=se_all, func=AF.Ln)
        scr = smalls.tile([128, Gc], f32)
        fin = smalls.tile([128, 1], f32)
        nc.vector.scalar_tensor_tensor(
            out=scr, in0=lse, scalar=1.0, in1=mkt,
            op0=OP.mult, op1=OP.mult, accum_out=fin)
        nc.sync.dma_start(out=outp[:, :], in_=fin)

    nc.compile()
    return nc


def host_prep(coord, coord_logits, disp, valid):
    """Host side: all O(B*H*W) terms + fp8 pixel-major repack of logits.

    Returns (in_maps, host_terms)."""
    coord = np.asarray(coord, np.float32)
    disp = np.asarray(disp, np.float32)
    valid = np.asarray(valid, bool)
    xl = np.asarray(coord_logits, np.float32)

    wcol = np.arange(W, dtype=np.float32)
    target = (wcol[None, None, :] - disp).astype(np.float32)
    mask = (valid & (disp < np.float32(192.0))).astype(np.float32)
    msum = float(mask.sum(dtype=np.float64)) + 1e-6
    l1 = float((np.abs(coord - target) * mask).sum(dtype=np.float64))

    labels = np.clip(target + np.float32(0.1 * W), np.float32(0.0),
                     np.float32(1.1 * W)).astype(np.float32)
    interval = np.float32(1.1 * W / 255.0)
    pos = (labels / interval).astype(np.float32)
    lb = np.floor(pos).astype(np.int32)
    hb = np.minimum(lb + 1, NBINS - 1)
    wh = (pos - lb.astype(np.float32)).astype(np.float32)
    x_lb = np.take_along_axis(xl, lb[:, None], axis=1)[:, 0]
    x_hb = np.take_along_axis(xl, hb[:, None], axis=1)[:, 0]
    interp = float((mask * ((1.0 - wh) * x_lb + wh * x_hb)).sum(
        dtype=np.float64))

    # fp8 cast once, then one full pixel-major transpose (B,H,W,C)
    x8 = xl.astype(ml_dtypes.float8_e3m4)
    x8 = np.ascontiguousarray(x8.transpose(0, 2, 3, 1))  # (B,H,W,C)
    mk16 = mask.astype(ml_dtypes.bfloat16)               # (B,H,W)

    in_maps = []
    for c in range(NCORES):
        r0, r1 = c * HC, (c + 1) * HC
        blk = np.ascontiguousarray(x8[:, r0:r1]).reshape(G, 128, NBINS)
        blk = np.ascontiguousarray(blk.transpose(1, 0, 2)).reshape(128, G * NBINS)
        m = np.ascontiguousarray(
            mk16[:, r0:r1].reshape(G, 128).transpose(1, 0))
        in_maps.append({"xl": blk, "mk": m})
    return in_maps, dict(msum=msum, l1=l1, interp=interp)


def combine(partials, terms):
    masklse = float(np.sum([np.asarray(p, np.float64).sum() for p in partials]))
    msum = terms["msum"]
    coord_loss = terms["l1"] / msum
    logits_loss = (masklse - terms["interp"]) / msum
    objective = 0.1 * coord_loss + logits_loss
    return (np.float32(objective), np.float32(coord_loss),
            np.float32(logits_loss))


_prog_cache = {}


def _get_program():
    key = tuple(sorted(CFG.items()))
    if key not in _prog_cache:
        _prog_cache[key] = build_program(CFG)
    return _prog_cache[key]


def kernel(coord, coord_logits, disp, valid):
    from concourse.bass_utils import run_bass_kernel_spmd

    nc = _get_program()
    in_maps, terms = host_prep(coord, coord_logits, disp, valid)
    res = run_bass_kernel_spmd(nc, in_maps, core_ids=list(range(NCORES)))
    partials = [r["outp"] for r in res.results]
    return combine(partials, terms)


# ---------------------------------------------------------------------------
# numpy model of the device program (for harness validation)
def sch_exp_np(x32):
    """numpy mirror of the device Schraudolph exp -> bf16 values."""
    t = (x32.astype(np.float32) * np.float32(SCH_A)
         + np.float32(SCH_B)).astype(np.float32)
    m = (t.view(np.uint32) & np.uint32(0xFFFF)).astype(np.uint16)
    return m.view(ml_dtypes.bfloat16).astype(np.float32)


def model_partials(cfg, in_map):
    Gc, C, GBt = cfg["G"], cfg["C"], cfg["GB"]
    PGt = cfg.get("PG", 0)
    GA = GBt - PGt
    x = in_map["xl"].astype(np.float32).reshape(128, Gc, C)
    se = np.exp(x).astype(ml_dtypes.bfloat16).astype(np.float32)
    if PGt:
        xt = x.reshape(128, Gc // GBt, GBt, C)
        st = se.reshape(128, Gc // GBt, GBt, C)
        st[:, :, GA:GBt] = sch_exp_np(xt[:, :, GA:GBt])
    w = C
    while w > 1:
        h = w // 2
        se = (se[:, :, 0:h] + se[:, :, h:w]).astype(
            ml_dtypes.bfloat16).astype(np.float32)
        w = h
    lse = np.log(se[:, :, 0])
    m = in_map["mk"].astype(np.float32)
    return (lse * m).sum(axis=1, dtype=np.float64).reshape(128, 1)
